# revision 11
# baseline (speedup 1.0000x reference)
"""Trainium2 Bass kernel for a GQA attention block (CodecTransformer).

Computes, for full inputs x[B=2,T=2048,D=2048], Wq[D,D], Wk/Wv[D,512], Wo[D,D]:
    y, k, v  (same as the fp32 jax reference: causal GQA attention with RoPE)

Sharding over 8 NeuronCores: core c = (b, g) with b = c // 4 (batch),
g = c % 4 (kv head group; 4 q heads + 1 kv head per group).  Each core
computes its group's partial y (Wo rows for its heads); the host sums the
4 partials per batch (the "Wo all-reduce" done host-side), and k/v cache
outputs come back directly per (b, g).

Device kernel per core (Tile framework, float32r matmuls):
  A) transpose x via PE -> xT; project qT/kT/vT (= W.T @ x, transposed
     layouts); RoPE applied in transposed layout via 64-partition shifted
     copies; k and v also PE-transposed back to natural [T, hd] for the
     cache outputs and the PV matmul.
  B) causal flash attention without max-subtraction (scores are O(+-8), so
     exp is safe in fp32): scoresT[s,t] strips via matmul, exp on ACT with
     the 1/sqrt(hd) scale folded in, 0/1 causal mask multiply on diagonal
     strips only, PV and ones-row denominator accumulated in PSUM, then
     normalize via a K=1 broadcast matmul of 1/denom.
  C) y_partial[t, :] = outT_norm.T @ Wo_g accumulated over the 4 heads.
"""
import numpy as np

import concourse.bass as bass
import concourse.mybir as mybir
import concourse.tile as tile
from concourse import bacc
from concourse.bass_utils import run_bass_kernel_spmd

F32R = mybir.dt.float32r
F32 = mybir.dt.float32
AF = mybir.ActivationFunctionType

P = 128           # partitions / head dim
T = 2048          # sequence length
D = 2048          # model dim
DC = D // P       # 16 contraction chunks
NH = 4            # q heads per core
TBLK = 512        # t block (matmul moving free dim)
NTB = T // TBLK   # 4 t blocks
NTC = T // P      # 16 t chunks
SCALE = 1.0 / np.sqrt(128.0)
ROPE_BASE = 10000.0


def _build_program(reps=1):
    nc = bacc.Bacc()
    x_d = nc.declare_dram_parameter("x", [T, D], F32R, isOutput=False)
    wq_d = nc.declare_dram_parameter("wq", [P, DC, NH * P], F32R, isOutput=False)
    wk_d = nc.declare_dram_parameter("wk", [P, DC, P], F32R, isOutput=False)
    wv_d = nc.declare_dram_parameter("wv", [P, DC, P], F32R, isOutput=False)
    wo_d = nc.declare_dram_parameter("wo", [P, NH, D], F32R, isOutput=False)
    id_d = nc.declare_dram_parameter("ident", [P, P], F32R, isOutput=False)
    onec_d = nc.declare_dram_parameter("onec", [P, 1], F32R, isOutput=False)
    oner_d = nc.declare_dram_parameter("oner", [1, P], F32R, isOutput=False)
    cost_d = nc.declare_dram_parameter("cost", [P, T], F32, isOutput=False)
    sintf_d = nc.declare_dram_parameter("sintf", [P, T], F32, isOutput=False)
    mask_d = nc.declare_dram_parameter("masks", [P, NH, TBLK], F32R, isOutput=False)
    yp_d = nc.declare_dram_parameter("yp", [T, D], F32, isOutput=True)
    ko_d = nc.declare_dram_parameter("k_out", [T, P], F32R, isOutput=True)
    vo_d = nc.declare_dram_parameter("v_out", [T, P], F32R, isOutput=True)

    from contextlib import ExitStack

    with tile.TileContext(nc) as tc, ExitStack() as top:
        cst = top.enter_context(tc.tile_pool(name="cst", bufs=1))
        per = top.enter_context(tc.tile_pool(name="per", bufs=1))
        id_sb = cst.tile([P, P], F32R)
        nc.sync.dma_start(out=id_sb[:], in_=id_d[:])
        onec_sb = cst.tile([P, 1], F32R)
        nc.sync.dma_start(out=onec_sb[:], in_=onec_d[:])
        oner_sb = cst.tile([1, P], F32R)
        nc.sync.dma_start(out=oner_sb[:], in_=oner_d[:])

        kT_sb = per.tile([P, T], F32R)          # RoPE'd kT  [hd, s]
        vN_sb = per.tile([P, NTC, P], F32R)     # v natural  [s%128, s//128, d]
        qT_sb = per.tile([P, NH, T], F32R)      # RoPE'd qT  [hd, h, t]

        for _rep in range(reps):
            _emit_body(nc, tc, x_d, wq_d, wk_d, wv_d, wo_d, id_sb, onec_sb,
                       oner_sb, cost_d, sintf_d, mask_d, yp_d, ko_d, vo_d,
                       kT_sb, vN_sb, qT_sb)

    nc.finalize()
    return nc


def _emit_body(nc, tc, x_d, wq_d, wk_d, wv_d, wo_d, id_sb, onec_sb, oner_sb,
               cost_d, sintf_d, mask_d, yp_d, ko_d, vo_d, kT_sb, vN_sb, qT_sb):
    from contextlib import ExitStack
    if True:
        # ---------------- Phase A: transpose x, project q/k/v, RoPE -------
        with ExitStack() as pa:
            wA = pa.enter_context(tc.tile_pool(name="wA", bufs=1))
            wq_sb = wA.tile([P, DC, NH * P], F32R)
            nc.sync.dma_start(out=wq_sb[:], in_=wq_d[:])
            wk_sb = wA.tile([P, DC, P], F32R)
            nc.sync.dma_start(out=wk_sb[:], in_=wk_d[:])
            wv_sb = wA.tile([P, DC, P], F32R)
            nc.sync.dma_start(out=wv_sb[:], in_=wv_d[:])
            cost_sb = wA.tile([P, T], F32)
            nc.sync.dma_start(out=cost_sb[:], in_=cost_d[:])
            sintf_sb = wA.tile([P, T], F32)
            nc.sync.dma_start(out=sintf_sb[:], in_=sintf_d[:])

            xn_pool = pa.enter_context(tc.tile_pool(name="xn", bufs=2))
            xT_pool = pa.enter_context(tc.tile_pool(name="xT", bufs=1))
            rp_pool = pa.enter_context(tc.tile_pool(name="rp", bufs=2))
            kv_pool = pa.enter_context(tc.tile_pool(name="kv", bufs=3))
            ps_tr = pa.enter_context(tc.tile_pool(name="pstr", bufs=2, space="PSUM"))
            ps_pj = pa.enter_context(tc.tile_pool(name="pspj", bufs=2, space="PSUM"))

            def rope_tr(src_ps, dst, tsl, nhp):
                """RoPE in transposed layout over [P, nhp, 512] (3D APs):
                dst = src*cosT + shift64(src)*sinTf (tables head-broadcast)."""
                shp = (P, nhp, TBLK)
                cosb = cost_sb[:, tsl].rearrange(
                    "p (o t) -> p o t", o=1).to_broadcast(shp)
                sinb = sintf_sb[:, tsl].rearrange(
                    "p (o t) -> p o t", o=1).to_broadcast(shp)
                tmp_t = rp_pool.tile([P, 2, TBLK], F32, tag="rtmp")
                tmp = tmp_t[:, :nhp, :]
                nc.vector.tensor_copy(tmp[0:64], src_ps[64:128])
                nc.vector.tensor_copy(tmp[64:128], src_ps[0:64])
                csd_t = rp_pool.tile([P, 2, TBLK], F32, tag="rcos")
                csd = csd_t[:, :nhp, :]
                nc.vector.tensor_mul(out=csd, in0=src_ps, in1=cosb)
                nc.vector.tensor_mul(out=tmp, in0=tmp, in1=sinb)
                nc.vector.tensor_add(out=dst, in0=csd, in1=tmp)

            for tb in range(NTB):
                tsl = slice(tb * TBLK, (tb + 1) * TBLK)
                xT_blk = xT_pool.tile([P, DC, TBLK], F32R, tag="xTblk")
                for tcc in range(4):
                    tg = tb * 4 + tcc
                    xn = xn_pool.tile([P, D], F32R, tag="xn")
                    nc.sync.dma_start(out=xn[:], in_=x_d[tg * P:(tg + 1) * P, :])
                    for half in range(2):
                        tps = ps_tr.tile([P, 8, P], F32R, tag="trps")
                        for j in range(8):
                            dc = half * 8 + j
                            nc.tensor.transpose(tps[:, j, :],
                                                xn[:, dc * P:(dc + 1) * P], id_sb[:])
                        nc.scalar.copy(
                            xT_blk[:, half * 8:(half + 1) * 8,
                                   tcc * P:(tcc + 1) * P], tps[:])

                # K and V projections (transposed) in one paired PSUM tile
                kvps = ps_pj.tile([P, 2, TBLK], F32, tag="projps")
                for dc in range(DC):
                    nc.tensor.matmul(kvps[:, 0, :], lhsT=wk_sb[:, dc, :],
                                     rhs=xT_blk[:, dc, :],
                                     start=(dc == 0), stop=(dc == DC - 1))
                for dc in range(DC):
                    nc.tensor.matmul(kvps[:, 1, :], lhsT=wv_sb[:, dc, :],
                                     rhs=xT_blk[:, dc, :],
                                     start=(dc == 0), stop=(dc == DC - 1))
                rope_tr(kvps[:, 0:1, :],
                        kT_sb[:, tsl].rearrange("p (o t) -> p o t", o=1), tsl, 1)
                vts = kv_pool.tile([P, TBLK], F32R, tag="vts")
                nc.scalar.copy(vts[:], kvps[:, 1, :])

                # k cache output: transpose back to [t, hd], batched copy + DMA
                ktr = ps_tr.tile([P, 8, P], F32R, tag="trps")
                for tcc in range(4):
                    tg = tb * 4 + tcc
                    nc.tensor.transpose(ktr[:, tcc, :],
                                        kT_sb[:, tg * P:(tg + 1) * P], id_sb[:])
                    nc.tensor.transpose(ktr[:, 4 + tcc, :],
                                        vts[:, tcc * P:(tcc + 1) * P], id_sb[:])
                kn = kv_pool.tile([P, 4, P], F32R, tag="kn")
                nc.scalar.copy(kn[:], ktr[:, 0:4, :])
                nc.sync.dma_start(
                    out=ko_d[tb * TBLK:(tb + 1) * TBLK, :].rearrange(
                        "(c p) d -> p c d", p=P),
                    in_=kn[:])
                nc.scalar.copy(vN_sb[:, tb * 4:(tb + 1) * 4, :], ktr[:, 4:8, :])
                nc.sync.dma_start(
                    out=vo_d[tb * TBLK:(tb + 1) * TBLK, :].rearrange(
                        "(c p) d -> p c d", p=P),
                    in_=vN_sb[:, tb * 4:(tb + 1) * 4, :])

                # Q projections (transposed) + RoPE, two heads per PSUM tile
                for hp in range(2):
                    qps = ps_pj.tile([P, 2, TBLK], F32, tag="projps")
                    for j in range(2):
                        h = hp * 2 + j
                        for dc in range(DC):
                            nc.tensor.matmul(
                                qps[:, j, :],
                                lhsT=wq_sb[:, dc, h * P:(h + 1) * P],
                                rhs=xT_blk[:, dc, :],
                                start=(dc == 0), stop=(dc == DC - 1))
                    rope_tr(qps[:], qT_sb[:, hp * 2:(hp + 1) * 2, tsl], tsl, 2)

        # ---------------- Phases B + C ------------------------------------
        with ExitStack() as pbc:
            mk = pbc.enter_context(tc.tile_pool(name="mk", bufs=1))
            mask_sb = mk.tile([P, NH, TBLK], F32R)
            nc.sync.dma_start(out=mask_sb[:], in_=mask_d[:])
            outT_sb = mk.tile([P, NH, T], F32R)   # normalized attn out, [hd, h, t]

            # Phase B: causal attention
            with ExitStack() as pb:
                eb = pb.enter_context(tc.tile_pool(name="eb", bufs=3))
                sm = pb.enter_context(tc.tile_pool(name="sm", bufs=2))
                ps_sc = pb.enter_context(tc.tile_pool(name="pssc", bufs=2, space="PSUM"))
                ps_pv = pb.enter_context(tc.tile_pool(name="pspv", bufs=1, space="PSUM"))
                ps_dn = pb.enter_context(tc.tile_pool(name="psdn", bufs=1, space="PSUM"))
                ps_bc = pb.enter_context(tc.tile_pool(name="psbc", bufs=1, space="PSUM"))
                for h in range(NH):
                    for tj in range(NTB):
                        tsl = slice(tj * TBLK, (tj + 1) * TBLK)
                        pvps = ps_pv.tile([P, TBLK], F32, tag="pv")
                        dnps = ps_dn.tile([1, TBLK], F32, tag="dn")
                        si_list = list(range(4 * tj + 3, -1, -1))  # diag strips first
                        nmm = len(si_list)
                        pairs = [si_list[i:i + 2] for i in range(0, nmm, 2)]
                        kk = 0
                        for pi, pr in enumerate(pairs):
                            scps = ps_sc.tile([P, 2, TBLK], F32, tag="sc")
                            for j, si in enumerate(pr):
                                nc.tensor.matmul(
                                    scps[:, j, :],
                                    lhsT=kT_sb[:, si * P:(si + 1) * P],
                                    rhs=qT_sb[:, h, tsl],
                                    start=True, stop=True)
                            ex = eb.tile([P, 2, TBLK], F32R, tag="ex")
                            nc.scalar.activation(ex[:], scps[:], AF.Exp, scale=SCALE)
                            # masks stored descending (j=0 -> r=3): the first
                            # two strips are exactly the diagonal ones
                            if pi < 2:
                                nc.vector.tensor_mul(
                                    out=ex[:], in0=ex[:],
                                    in1=mask_sb[:, pi * 2:(pi + 1) * 2, :])
                            for j, si in enumerate(pr):
                                nc.tensor.matmul(pvps[:], lhsT=vN_sb[:, si, :],
                                                 rhs=ex[:, j, :],
                                                 start=(kk == 0), stop=(kk == nmm - 1))
                                nc.tensor.matmul(dnps[:], lhsT=onec_sb[:],
                                                 rhs=ex[:, j, :],
                                                 start=(kk == 0), stop=(kk == nmm - 1))
                                kk += 1
                        rec = sm.tile([1, TBLK], F32R, tag="rec")
                        with nc.allow_low_precision(reason="f32r is fp32-width"):
                            nc.vector.reciprocal(rec[:], dnps[:])
                        bcps = ps_bc.tile([P, TBLK], F32, tag="bc")
                        nc.tensor.matmul(bcps[:], lhsT=oner_sb[:], rhs=rec[:],
                                         start=True, stop=True)
                        bcs = sm.tile([P, TBLK], F32R, tag="bcs")
                        nc.scalar.copy(bcs[:], bcps[:])
                        nc.vector.tensor_mul(out=outT_sb[:, h, tsl],
                                             in0=pvps[:], in1=bcs[:])

            # Phase C: y_partial = outT_norm.T @ Wo_g
            with ExitStack() as pc:
                woP = pc.enter_context(tc.tile_pool(name="woP", bufs=1))
                wo_sb = woP.tile([P, NH, D], F32R)
                nc.sync.dma_start(out=wo_sb[:], in_=wo_d[:])
                yb = pc.enter_context(tc.tile_pool(name="yb", bufs=2))
                ps_y = pc.enter_context(tc.tile_pool(name="psy", bufs=2, space="PSUM"))
                for tg in range(NTC):
                    yps = ps_y.tile([P, D], F32, tag="y")
                    for nb in range(NTB):
                        for h in range(NH):
                            nc.tensor.matmul(
                                yps[:, nb * TBLK:(nb + 1) * TBLK],
                                lhsT=outT_sb[:, h, tg * P:(tg + 1) * P],
                                rhs=wo_sb[:, h, nb * TBLK:(nb + 1) * TBLK],
                                start=(h == 0), stop=(h == NH - 1))
                    ysb = yb.tile([P, D], F32, tag="ysb")
                    nc.scalar.copy(ysb[:], yps[:])
                    nc.sync.dma_start(out=yp_d[tg * P:(tg + 1) * P, :], in_=ysb[:])


def _host_tables():
    inv_freq = 1.0 / (ROPE_BASE ** (np.arange(0, P, 2, dtype=np.float32) / P))
    t = np.arange(T, dtype=np.float32)
    freqs = np.outer(t, inv_freq)                      # [T, 64]
    emb = np.concatenate([freqs, freqs], axis=-1)      # [T, 128]
    cos = np.cos(emb).astype(np.float32)
    sin = np.sin(emb).astype(np.float32)
    cosT = np.ascontiguousarray(cos.T)                 # [128, T]
    sinT = sin.T
    sintf = np.concatenate([-sinT[:64], sinT[64:]], axis=0).astype(np.float32)
    sintf = np.ascontiguousarray(sintf)
    # causal masks for diagonal strips: mask[p, r, f] = 1 if f >= p + r*128
    f = np.arange(TBLK)[None, None, :]
    p = np.arange(P)[:, None, None]
    r = 3 - np.arange(NH)[None, :, None]
    masks = (f >= p + r * P).astype(np.float32)
    masks = np.ascontiguousarray(masks)                # [128, 4, 512]
    return cosT, sintf, masks


_NC_CACHE = {}


def _get_program():
    if "nc" not in _NC_CACHE:
        _NC_CACHE["nc"] = _build_program()
    return _NC_CACHE["nc"]


def _make_in_maps(np_inputs):
    x = np.asarray(np_inputs["x"], dtype=np.float32)
    Wq = np.asarray(np_inputs["Wq"], dtype=np.float32)
    Wk = np.asarray(np_inputs["Wk"], dtype=np.float32)
    Wv = np.asarray(np_inputs["Wv"], dtype=np.float32)
    Wo = np.asarray(np_inputs["Wo"], dtype=np.float32)

    cosT, sintf, masks = _host_tables()
    ident = np.eye(P, dtype=np.float32)
    onec = np.ones((P, 1), dtype=np.float32)
    oner = np.ones((1, P), dtype=np.float32)

    in_maps = []
    for c in range(8):
        b, g = c // 4, c % 4
        wq_g = Wq[:, g * 512:(g + 1) * 512]
        in_maps.append({
            "x": np.ascontiguousarray(x[b]),
            "wq": np.ascontiguousarray(
                wq_g.reshape(DC, P, NH * P).transpose(1, 0, 2)),
            "wk": np.ascontiguousarray(
                Wk[:, g * P:(g + 1) * P].reshape(DC, P, P).transpose(1, 0, 2)),
            "wv": np.ascontiguousarray(
                Wv[:, g * P:(g + 1) * P].reshape(DC, P, P).transpose(1, 0, 2)),
            "wo": np.ascontiguousarray(
                Wo[g * 512:(g + 1) * 512, :].reshape(NH, P, D).transpose(1, 0, 2)),
            "ident": ident, "onec": onec, "oner": oner,
            "cost": cosT, "sintf": sintf, "masks": masks,
        })
    return in_maps


def kernel(x, Wq, Wk, Wv, Wo, _trace=False, _trace_kwargs=None):
    nc = _get_program()
    in_maps = _make_in_maps(dict(x=x, Wq=Wq, Wk=Wk, Wv=Wv, Wo=Wo))

    kw = {}
    if _trace:
        kw["trace"] = True
        kw.update(_trace_kwargs or {})
    res = run_bass_kernel_spmd(nc, in_maps, list(range(8)), **kw)
    outs = res.results

    y = np.empty((2, T, D), dtype=np.float32)
    k = np.empty((2, NH, T, P), dtype=np.float32)
    v = np.empty((2, NH, T, P), dtype=np.float32)
    for b in range(2):
        acc = outs[b * 4]["yp"].astype(np.float32)
        for g in range(1, 4):
            acc = acc + outs[b * 4 + g]["yp"]
        y[b] = acc
        for g in range(4):
            k[b, g] = outs[b * 4 + g]["k_out"]
            v[b, g] = outs[b * 4 + g]["v_out"]
    if _trace:
        return (y, k, v), res
    return y, k, v
